# revision 9
# baseline (speedup 1.0000x reference)
"""Cross-attention + output projection + residual + GroupNorm on 8 NeuronCores.

Problem (hardcoded): B=4, C=256, H=W=48 (N=2304 pixels), 4 heads x 64 dim,
GroupNorm with 32 groups of 8 channels, eps=1e-5.

Sharding: 2 cores per batch element; each core handles one half of the
query pixels (1152) for all 4 heads.  K/V are computed for the full pixel
range on both cores of a pair (duplicated, cheap).  The only cross-core
communication is a 2KB AllReduce of per-channel (sum, sumsq) GroupNorm
partial statistics between the two cores of each pair.

Per-core math (channel-major layouts, pixels on the free axis):
  q = WqT.T @ xh + bq                  [256, 1152]
  k = WkT.T @ ctx + bk                 [256, 2304]
  vT[j, c] = (ctx.T @ WvT)[j, c]       [2304, 256]  (+ ones column per head)
  per head h: sT[j, i] = k_h[d, j].T-contracted with q_h[d, i]  (PE, K=64)
              eT = exp(0.125 * sT)     (ACT, no max subtraction needed:
                                        scores are ~N(0,1) for this problem)
              avT[d_aug, i] = sum_j vT_aug[j, d_aug] * eT[j, i]   (PE, K=128)
              row 64 of avT is the softmax denominator (ones column)
              ao_h = avT[0:64] * (1/den) broadcast                (DVE)
  y = WoT.T @ (ao + bv) + bo + xh      [256, 1152]
  per-channel partial stats (sum, sumsq) -> pair AllReduce -> group stats
  via 0/1 selection matmuls -> y = (y - mean) * rstd * gamma + beta
"""

import sys

if "/opt/trn_rl_repo" not in sys.path:
    sys.path.insert(0, "/opt/trn_rl_repo")

import numpy as np

import concourse.bass as bass
import concourse.mybir as mybir
import concourse.tile as tile
from concourse import bacc
from concourse.bass_utils import run_bass_kernel_spmd

F32 = mybir.dt.float32
AF = mybir.ActivationFunctionType
ALU = mybir.AluOpType

B, C, HW = 4, 256, 2304
NH, HD = 4, 64
NHALF = HW // 2  # 1152 query pixels per core
SCALE = HD ** -0.5  # 0.125
GSIZE = 8  # channels per GroupNorm group
EPS = 1e-5
GN_COUNT = GSIZE * HW  # elements per group per batch (after pair AllReduce)

_CACHE = {}


def _mm_slices(total, step=512):
    return [(s, min(s + step, total)) for s in range(0, total, step)]


def _bcast_ap(ap, nparts):
    """Partition-broadcast view of a single-partition AP."""
    return bass.AP(tensor=ap.tensor, offset=ap.offset, ap=[[0, nparts]] + list(ap.ap)[1:])


def _finalize(nc):
    """compile() leaves 3+-wait Matmults that walrus rejects ("Too many sync
    wait commands" on the S3_LW struct); a second compile pass — run here via
    finalize() — splits them onto EventSemaphores.  Verify that it worked."""
    nc.compile()
    nc.finalize()
    for fn in nc.m.functions:
        for bb in fn.blocks:
            for inst in bb.instructions:
                si = inst.sync_info
                if isinstance(inst, mybir.InstMatmult) and si is not None:
                    assert len(si.on_wait or []) <= 2, (inst.name, si.on_wait)


def _build():
    nc = bacc.Bacc("TRN2", target_bir_lowering=False, debug=False, num_devices=8)

    xh_d = nc.dram_tensor("xh", [C, NHALF], F32, kind="ExternalInput").ap()
    ctx_d = nc.dram_tensor("ctx", [C, HW], F32, kind="ExternalInput").ap()
    w_d = {
        nm: nc.dram_tensor(nm, [C, C], F32, kind="ExternalInput").ap()
        for nm in ("wqT", "wkT", "wvT", "woT")
    }
    b_d = {
        nm: nc.dram_tensor(nm, [C, 1], F32, kind="ExternalInput").ap()
        for nm in ("bq", "bk", "bv", "bo", "gamma", "beta")
    }
    gsel_d = nc.dram_tensor("gsel", [128, 16], F32, kind="ExternalInput").ap()
    gselT_d = nc.dram_tensor("gselT", [16, 128], F32, kind="ExternalInput").ap()
    yh_d = nc.dram_tensor("yh", [C, NHALF], F32, kind="ExternalOutput").ap()

    NJT = HW // 128  # 18 key tiles of 128

    with tile.TileContext(nc) as tc:
        with (
            tc.tile_pool(name="const", bufs=1) as const,
            tc.tile_pool(name="main", bufs=1) as main,
            tc.tile_pool(name="small", bufs=4) as small,
            tc.tile_pool(name="dram", bufs=4, space="DRAM") as dram,
        ):
            # ---- constants ----
            w_sb = {}
            for nm in ("wqT", "wkT", "wvT", "woT"):
                t = const.tile([128, 2, C], F32, tag=nm)
                nc.sync.dma_start(out=t, in_=w_d[nm].rearrange("(k p) o -> p k o", p=128))
                w_sb[nm] = t
            b_sb = {}
            for nm in ("bq", "bk", "bv", "bo", "gamma", "beta"):
                t = const.tile([128, 2], F32, tag=nm)
                nc.sync.dma_start(out=t, in_=b_d[nm].rearrange("(k p) one -> p (k one)", p=128))
                b_sb[nm] = t
            gsel_sb = const.tile([128, 16], F32, tag="gsel")
            nc.sync.dma_start(out=gsel_sb, in_=gsel_d)
            gselT_sb = const.tile([16, 128], F32, tag="gselT")
            nc.sync.dma_start(out=gselT_sb, in_=gselT_d)
            eps_sb = const.tile([16, 1], F32, tag="eps")
            nc.vector.memset(eps_sb, EPS)

            # ---- activations (long-lived) ----
            xh_sb = main.tile([128, 2, NHALF], F32, tag="xh")
            for k in range(2):
                for s, e in _mm_slices(NHALF):
                    nc.sync.dma_start(
                        out=xh_sb[:, k, s:e],
                        in_=xh_d.rearrange("(k p) i -> p k i", p=128)[:, k, s:e],
                    )
            q_sb = main.tile([128, 2, NHALF], F32, tag="q")
            k_sb = main.tile([128, 2, HW], F32, tag="k")
            vT_sb = main.tile([128, NJT, NH * (HD + 1)], F32, tag="vT")
            ao_sb = main.tile([128, 2, NHALF], F32, tag="ao")
            y_sb = main.tile([128, 2, NHALF], F32, tag="y")
            sums = small.tile([128, 2], F32, tag="sums", bufs=1)
            sumsq = small.tile([128, 2], F32, tag="sumsq", bufs=1)
            scr = main.tile([128, NHALF], F32, tag="scr")  # TTR throwaway out

            # ones columns of vT (one per head, strided over j-tiles)
            for h in range(NH):
                c0 = h * (HD + 1) + HD
                nc.vector.memset(vT_sb[:, :, c0 : c0 + 1], 1.0)

            # ---- projections ----
            with (
                tc.tile_pool(name="ctxp", bufs=1) as ctxp,
                tc.tile_pool(name="pp", bufs=2, space="PSUM") as pp,
            ):
                ctx_sb = ctxp.tile([128, 2, HW], F32, tag="ctx")
                for k in range(2):
                    for jh in range(2):
                        for s, e in _mm_slices(NHALF):
                            o = jh * NHALF
                            nc.sync.dma_start(
                                out=ctx_sb[:, k, o + s : o + e],
                                in_=ctx_d.rearrange("(k p) j -> p k j", p=128)[
                                    :, k, o + s : o + e
                                ],
                            )

                # Q: [o_grp 128, 1152]
                for g in range(2):
                    ps = pp.tile([128, NHALF], F32, tag="qk")
                    for k in range(2):
                        lhsT = w_sb["wqT"][:, k, g * 128 : (g + 1) * 128]
                        for s, e in _mm_slices(NHALF):
                            nc.tensor.matmul(
                                ps[:, s:e], lhsT, xh_sb[:, k, s:e],
                                start=(k == 0), stop=(k == 1),
                            )
                    nc.vector.tensor_scalar_add(
                        out=q_sb[:, g, :], in0=ps, scalar1=b_sb["bq"][:, g : g + 1]
                    )

                # K: [o_grp 128, 2304] in two j-halves
                for g in range(2):
                    for jh in range(2):
                        ps = pp.tile([128, NHALF], F32, tag="qk")
                        for k in range(2):
                            lhsT = w_sb["wkT"][:, k, g * 128 : (g + 1) * 128]
                            for s, e in _mm_slices(NHALF):
                                nc.tensor.matmul(
                                    ps[:, s:e], lhsT,
                                    ctx_sb[:, k, jh * NHALF + s : jh * NHALF + e],
                                    start=(k == 0), stop=(k == 1),
                                )
                        nc.vector.tensor_scalar_add(
                            out=k_sb[:, g, jh * NHALF : (jh + 1) * NHALF],
                            in0=ps, scalar1=b_sb["bk"][:, g : g + 1],
                        )

                # V transposed: [j_tile 128, 256] per tile
                for jt in range(NJT):
                    ps = pp.tile([128, C], F32, tag="vp")
                    for k in range(2):
                        nc.tensor.matmul(
                            ps, ctx_sb[:, k, jt * 128 : (jt + 1) * 128],
                            w_sb["wvT"][:, k, :],
                            start=(k == 0), stop=(k == 1),
                        )
                    for h in range(NH):
                        nc.vector.tensor_copy(
                            out=vT_sb[:, jt, h * (HD + 1) : h * (HD + 1) + HD],
                            in_=ps[:, h * HD : (h + 1) * HD],
                        )

            # ---- attention ----
            with (
                tc.tile_pool(name="expp", bufs=20) as expp,
                tc.tile_pool(name="rbp", bufs=3) as rbp,
                tc.tile_pool(name="scp", bufs=2, space="PSUM") as scp,
                tc.tile_pool(name="avp", bufs=2, space="PSUM") as avp,
            ):
                for h in range(NH):
                    g, off = h // 2, (h % 2) * HD
                    kv_col = h * (HD + 1)
                    exp_tiles = []
                    for jt in range(NJT):
                        ps = scp.tile([128, NHALF], F32, tag="sc")
                        lhsT = k_sb[off : off + HD, g, jt * 128 : (jt + 1) * 128]
                        rhs = q_sb[off : off + HD, g, :]
                        for s, e in _mm_slices(NHALF):
                            nc.tensor.matmul(ps[:, s:e], lhsT, rhs[:, s:e],
                                             start=True, stop=True)
                        ex = expp.tile([128, NHALF], F32, tag="exp")
                        nc.scalar.activation(out=ex, in_=ps, func=AF.Exp, scale=SCALE)
                        exp_tiles.append(ex)

                    for s, e in _mm_slices(NHALF):
                        av = avp.tile([HD + 1, 512], F32, tag="av")
                        for jt in range(NJT):
                            nc.tensor.matmul(
                                av[:, : e - s], vT_sb[:, jt, kv_col : kv_col + HD + 1],
                                exp_tiles[jt][:, s:e],
                                start=(jt == 0), stop=(jt == NJT - 1),
                            )
                        rd = small.tile([1, 512], F32, tag="rd")
                        nc.vector.reciprocal(out=rd[:, : e - s], in_=av[HD : HD + 1, : e - s])
                        rdd = dram.tile([1, 512], F32, tag="rdd")
                        nc.sync.dma_start(out=rdd[:, : e - s], in_=rd[:, : e - s])
                        rb = rbp.tile([HD, 512], F32, tag="rb")
                        nc.sync.dma_start(out=rb[:, : e - s], in_=_bcast_ap(rdd[:1, : e - s], HD))
                        nc.vector.tensor_tensor(
                            out=ao_sb[off : off + HD, g, s:e],
                            in0=av[:HD, : e - s], in1=rb[:, : e - s], op=ALU.mult,
                        )

            # ---- output projection + residual + GroupNorm ----
            with (
                tc.tile_pool(name="wop", bufs=2, space="PSUM") as wop,
                tc.tile_pool(name="gnp", bufs=1, space="PSUM") as gnp,
            ):
                for g in range(2):  # v bias (usually zero, kept for generality)
                    nc.vector.tensor_scalar_add(
                        out=ao_sb[:, g, :], in0=ao_sb[:, g, :],
                        scalar1=b_sb["bv"][:, g : g + 1],
                    )
                for g in range(2):
                    ps = wop.tile([128, NHALF], F32, tag="wo")
                    for k in range(2):
                        lhsT = w_sb["woT"][:, k, g * 128 : (g + 1) * 128]
                        for s, e in _mm_slices(NHALF):
                            nc.tensor.matmul(ps[:, s:e], lhsT, ao_sb[:, k, s:e],
                                             start=(k == 0), stop=(k == 1))
                    nc.vector.tensor_scalar_add(
                        out=y_sb[:, g, :], in0=ps, scalar1=b_sb["bo"][:, g : g + 1]
                    )
                    nc.vector.tensor_add(
                        out=y_sb[:, g, :], in0=y_sb[:, g, :], in1=xh_sb[:, g, :]
                    )
                    nc.vector.reduce_sum(
                        out=sums[:, g : g + 1], in_=y_sb[:, g, :],
                        axis=mybir.AxisListType.X,
                    )
                    nc.vector.tensor_mul(out=scr, in0=y_sb[:, g, :], in1=y_sb[:, g, :])
                    nc.vector.reduce_sum(
                        out=sumsq[:, g : g + 1], in_=scr, axis=mybir.AxisListType.X
                    )

                # pair AllReduce of per-channel (sum, sumsq)
                gn_in = dram.tile([C, 2], F32, tag="gnin", bufs=1)
                gn_out = dram.tile([C, 2], F32, tag="gnout", bufs=1)
                gn_in_r = gn_in.rearrange("(k p) s -> p k s", p=128)
                for k in range(2):
                    nc.sync.dma_start(out=gn_in_r[:, k, 0:1], in_=sums[:, k : k + 1])
                    nc.sync.dma_start(out=gn_in_r[:, k, 1:2], in_=sumsq[:, k : k + 1])
                nc.gpsimd.collective_compute(
                    "AllReduce", ALU.add,
                    replica_groups=[[0, 1], [2, 3], [4, 5], [6, 7]],
                    ins=[gn_in.opt()], outs=[gn_out.opt()],
                )
                gs_sb = small.tile([128, 2, 2], F32, tag="gs", bufs=1)
                nc.sync.dma_start(out=gs_sb, in_=gn_out.rearrange("(k p) s -> p k s", p=128))

                # group totals via 0/1 selection matmul: [16 local groups, (sum,sumsq)]
                gtot = small.tile([16, 2, 2], F32, tag="gtot", bufs=1)
                for k in range(2):
                    gp = gnp.tile([16, 2], F32, tag="gp")
                    nc.tensor.matmul(gp, gsel_sb, gs_sb[:, k, :], start=True, stop=True)
                    nc.vector.tensor_copy(out=gtot[:, k, :], in_=gp)
                mean_g = small.tile([16, 2], F32, tag="meang", bufs=1)
                var_g = small.tile([16, 2], F32, tag="varg", bufs=1)
                nc.scalar.mul(out=mean_g, in_=gtot[:, :, 0], mul=1.0 / GN_COUNT)
                nc.scalar.mul(out=var_g, in_=gtot[:, :, 1], mul=1.0 / GN_COUNT)
                m2 = small.tile([16, 2], F32, tag="m2", bufs=1)
                nc.vector.tensor_mul(out=m2, in0=mean_g, in1=mean_g)
                nc.vector.tensor_tensor(out=var_g, in0=var_g, in1=m2, op=ALU.subtract)
                nc.scalar.activation(out=var_g, in_=var_g, func=AF.Sqrt, bias=eps_sb)
                nc.vector.reciprocal(out=var_g, in_=var_g)  # rstd [16, 2]

                # broadcast group stats to channels: [128, 2] via gselT matmul
                mean_c = small.tile([128, 2], F32, tag="meanc", bufs=1)
                rstd_c = small.tile([128, 2], F32, tag="rstdc", bufs=1)
                for src, dst in ((mean_g, mean_c), (var_g, rstd_c)):
                    gp = gnp.tile([128, 2], F32, tag="gb")
                    nc.tensor.matmul(gp, gselT_sb, src, start=True, stop=True)
                    nc.vector.tensor_copy(out=dst, in_=gp)

                yr = yh_d.rearrange("(k p) i -> p k i", p=128)
                for g in range(2):
                    nc.vector.tensor_scalar(
                        out=y_sb[:, g, :], in0=y_sb[:, g, :],
                        scalar1=mean_c[:, g : g + 1], scalar2=rstd_c[:, g : g + 1],
                        op0=ALU.subtract, op1=ALU.mult,
                    )
                    nc.vector.tensor_scalar(
                        out=y_sb[:, g, :], in0=y_sb[:, g, :],
                        scalar1=b_sb["gamma"][:, g : g + 1],
                        scalar2=b_sb["beta"][:, g : g + 1],
                        op0=ALU.mult, op1=ALU.add,
                    )
                    for s, e in _mm_slices(NHALF, 576):
                        nc.sync.dma_start(out=yr[:, g, s:e], in_=y_sb[:, g, s:e])

    _finalize(nc)
    return nc


def _get_nc():
    if "nc" not in _CACHE:
        _CACHE["nc"] = _build()
    return _CACHE["nc"]


def kernel(x, context, Wq, bq, Wk, bk, Wv, bv, Wo, bo, gamma, beta):
    x = np.asarray(x, np.float32)
    context = np.asarray(context, np.float32)
    xr = np.ascontiguousarray(x.reshape(B, C, HW))
    cr = np.ascontiguousarray(context.reshape(B, C, HW))

    gsel = np.zeros((128, 16), np.float32)
    gsel[np.arange(128), np.arange(128) // GSIZE] = 1.0

    shared = {
        "wqT": np.ascontiguousarray(np.asarray(Wq, np.float32).T),
        "wkT": np.ascontiguousarray(np.asarray(Wk, np.float32).T),
        "wvT": np.ascontiguousarray(np.asarray(Wv, np.float32).T),
        "woT": np.ascontiguousarray(np.asarray(Wo, np.float32).T),
        "bq": np.asarray(bq, np.float32).reshape(C, 1),
        "bk": np.asarray(bk, np.float32).reshape(C, 1),
        "bv": np.asarray(bv, np.float32).reshape(C, 1),
        "bo": np.asarray(bo, np.float32).reshape(C, 1),
        "gamma": np.asarray(gamma, np.float32).reshape(C, 1),
        "beta": np.asarray(beta, np.float32).reshape(C, 1),
        "gsel": gsel,
        "gselT": np.ascontiguousarray(gsel.T),
    }
    in_maps = []
    for core in range(8):
        b, half = core // 2, core % 2
        m = dict(shared)
        m["xh"] = np.ascontiguousarray(xr[b, :, half * NHALF : (half + 1) * NHALF])
        m["ctx"] = cr[b]
        in_maps.append(m)

    nc = _get_nc()
    res = run_bass_kernel_spmd(nc, in_maps, core_ids=list(range(8)))

    out = np.empty((B, C, HW), np.float32)
    for core in range(8):
        b, half = core // 2, core % 2
        out[b, :, half * NHALF : (half + 1) * NHALF] = res.results[core]["yh"]
    return out.reshape(x.shape)


# revision 12
# speedup vs baseline: 1.7649x; 1.7649x over previous
"""Cross-attention + output projection + residual + GroupNorm on 8 NeuronCores.

Problem (hardcoded): B=4, C=256, H=W=48 (N=2304 pixels), 4 heads x 64 dim,
GroupNorm with 32 groups of 8 channels, eps=1e-5.

Sharding: 2 cores per batch element; each core handles one half of the
query pixels (1152) for all 4 heads.  K/V are computed for the full pixel
range on both cores of a pair (duplicated, cheap).  The only cross-core
communication is a 2KB AllReduce of per-channel (sum, sumsq) GroupNorm
partial statistics between the two cores of each pair.

Per-core math (channel-major layouts, pixels on the free axis):
  q = WqT.T @ xh + bq                  [256, 1152]   (fp32r matmul)
  k = WkT.T @ ctx + bk                 [256, 2304]   (fp32r matmul)
  vT[j, c] = (ctx.T @ WvT)[j, c]       [2304, 256]   (fp32r matmul,
                                        + ones column per head, bf16)
  per head h (bf16 matmuls):
    sT[j, i] = sum_d k_h[d, j] q_h[d, i]     (PE, K=64)
    eT = exp(0.125 * sT)  -> bf16            (ACT; no max subtraction
                                              needed: scores ~ N(0,1))
    av[i, d_aug] = sum_j eT[j, i] vT_aug[j, d_aug]   (PE, K=128)
    column 64 of av is the softmax denominator (ones column), so the
    normalization scalars are per-partition: ao_n = av[:, :64] / av[:, 64]
    ao[c, i] via PE transpose of ao_n
  y = WoT.T @ (ao + bv) + bo + xh      [256, 1152]   (fp32r matmul)
  per-channel partial stats (sum, sumsq) -> pair AllReduce -> group stats
  via 0/1 selection matmuls -> y = (y - mean) * rstd * gamma + beta
"""

import sys

if "/opt/trn_rl_repo" not in sys.path:
    sys.path.insert(0, "/opt/trn_rl_repo")

import numpy as np

import concourse.bass as bass
import concourse.mybir as mybir
import concourse.tile as tile
from concourse import bacc
from concourse.bass_utils import run_bass_kernel_spmd
from concourse.masks import make_identity

F32 = mybir.dt.float32
F32R = mybir.dt.float32r
BF16 = mybir.dt.bfloat16
AF = mybir.ActivationFunctionType
ALU = mybir.AluOpType

B, C, HW = 4, 256, 2304
NH, HD = 4, 64
NHALF = HW // 2  # 1152 query pixels per core
SCALE = HD ** -0.5  # 0.125
GSIZE = 8  # channels per GroupNorm group
EPS = 1e-5
GN_COUNT = GSIZE * HW  # elements per group per batch (after pair AllReduce)

_CACHE = {}


def _mm_slices(total, step=512):
    return [(s, min(s + step, total)) for s in range(0, total, step)]


def _finalize(nc):
    """compile() leaves 3+-wait Matmults that walrus rejects ("Too many sync
    wait commands" on the S3_LW struct); a second compile pass — run here via
    finalize() — splits them onto EventSemaphores.  Verify that it worked."""
    nc.compile()
    nc.finalize()
    for fn in nc.m.functions:
        for bb in fn.blocks:
            for inst in bb.instructions:
                si = inst.sync_info
                if isinstance(inst, mybir.InstMatmult) and si is not None:
                    assert len(si.on_wait or []) <= 2, (inst.name, si.on_wait)


def _build():
    nc = bacc.Bacc("TRN2", target_bir_lowering=False, debug=False, num_devices=8)

    xh_d = nc.dram_tensor("xh", [C, NHALF], F32, kind="ExternalInput").ap()
    ctx_d = nc.dram_tensor("ctx", [C, HW], F32R, kind="ExternalInput").ap()
    xhr_d = nc.dram_tensor("xhr", [C, NHALF], F32R, kind="ExternalInput").ap()
    w_d = {
        nm: nc.dram_tensor(nm, [C, C], F32R if nm != "woT" else F32,
                           kind="ExternalInput").ap()
        for nm in ("wqT", "wkT", "wvT", "woT")
    }
    b_d = {
        nm: nc.dram_tensor(nm, [C, 1], F32, kind="ExternalInput").ap()
        for nm in ("bq", "bk", "bv", "bo", "gamma", "beta")
    }
    gsel_d = nc.dram_tensor("gsel", [128, 16], F32, kind="ExternalInput").ap()
    gselT_d = nc.dram_tensor("gselT", [16, 128], F32, kind="ExternalInput").ap()
    yh_d = nc.dram_tensor("yh", [C, NHALF], F32, kind="ExternalOutput").ap()

    NJT = HW // 128  # 18 key tiles of 128

    with tile.TileContext(nc) as tc:
        with (
            tc.tile_pool(name="const", bufs=1) as const,
            tc.tile_pool(name="main", bufs=1) as main,
            tc.tile_pool(name="small", bufs=4) as small,
            tc.tile_pool(name="dram", bufs=2, space="DRAM") as dram,
        ):
            # ---- constants ----
            w_sb = {}
            for nm in ("wqT", "wkT", "wvT", "woT"):
                t = const.tile([128, 2, C], F32R if nm != "woT" else F32, tag=nm)
                nc.sync.dma_start(out=t, in_=w_d[nm].rearrange("(k p) o -> p k o", p=128))
                w_sb[nm] = t
            b_sb = {}
            for nm in ("bq", "bk", "bv", "bo", "gamma", "beta"):
                t = const.tile([128, 2], F32, tag=nm)
                nc.sync.dma_start(out=t, in_=b_d[nm].rearrange("(k p) one -> p (k one)", p=128))
                b_sb[nm] = t
            gsel_sb = const.tile([128, 16], F32, tag="gsel")
            nc.sync.dma_start(out=gsel_sb, in_=gsel_d)
            gselT_sb = const.tile([16, 128], F32, tag="gselT")
            nc.sync.dma_start(out=gselT_sb, in_=gselT_d)
            eps_sb = const.tile([16, 1], F32, tag="eps")
            nc.vector.memset(eps_sb, EPS)
            ident = const.tile([128, 128], F32, tag="ident")
            make_identity(nc, ident)

            # ---- activations (long-lived) ----
            xh_sb = main.tile([128, 2, NHALF], F32, tag="xh")
            for k in range(2):
                for s, e in _mm_slices(NHALF):
                    nc.sync.dma_start(
                        out=xh_sb[:, k, s:e],
                        in_=xh_d.rearrange("(k p) i -> p k i", p=128)[:, k, s:e],
                    )
            q_sb = main.tile([128, 2, NHALF], BF16, tag="q")
            k_sb = main.tile([128, 2, HW], BF16, tag="k")
            vT_sb = main.tile([128, NJT, NH * (HD + 1)], BF16, tag="vT")
            ao_sb = main.tile([128, 2, NHALF], F32, tag="ao")
            y_sb = main.tile([128, 2, NHALF], F32, tag="y")
            sums = small.tile([128, 2], F32, tag="sums", bufs=1)
            sumsq = small.tile([128, 2], F32, tag="sumsq", bufs=1)
            scr = main.tile([128, NHALF], F32, tag="scr")

            # ones columns of vT (one per head, strided over j-tiles)
            for h in range(NH):
                c0 = h * (HD + 1) + HD
                nc.vector.memset(vT_sb[:, :, c0 : c0 + 1], 1.0)

            # ---- projections (fp32r) ----
            with (
                tc.tile_pool(name="ctxp", bufs=1) as ctxp,
                tc.tile_pool(name="pp", bufs=2, space="PSUM") as pp,
            ):
                ctx_sb = ctxp.tile([128, 2, HW], F32R, tag="ctx")
                xhr_sb = ctxp.tile([128, 2, NHALF], F32R, tag="xhr")
                for k in range(2):
                    for s, e in _mm_slices(NHALF):
                        nc.sync.dma_start(
                            out=xhr_sb[:, k, s:e],
                            in_=xhr_d.rearrange("(k p) i -> p k i", p=128)[:, k, s:e],
                        )
                for k in range(2):
                    for jh in range(2):
                        for s, e in _mm_slices(NHALF):
                            o = jh * NHALF
                            nc.sync.dma_start(
                                out=ctx_sb[:, k, o + s : o + e],
                                in_=ctx_d.rearrange("(k p) j -> p k j", p=128)[
                                    :, k, o + s : o + e
                                ],
                            )

                # Q: [o_grp 128, 1152] -> q_sb bf16
                for g in range(2):
                    ps = pp.tile([128, NHALF], F32, tag="qk")
                    for k in range(2):
                        lhsT = w_sb["wqT"][:, k, g * 128 : (g + 1) * 128]
                        for s, e in _mm_slices(NHALF):
                            nc.tensor.matmul(
                                ps[:, s:e], lhsT, xhr_sb[:, k, s:e],
                                start=(k == 0), stop=(k == 1),
                            )
                    nc.vector.tensor_scalar_add(
                        out=q_sb[:, g, :], in0=ps, scalar1=b_sb["bq"][:, g : g + 1]
                    )

                # K: [o_grp 128, 2304] in two j-halves -> k_sb bf16
                for g in range(2):
                    for jh in range(2):
                        ps = pp.tile([128, NHALF], F32, tag="qk")
                        for k in range(2):
                            lhsT = w_sb["wkT"][:, k, g * 128 : (g + 1) * 128]
                            for s, e in _mm_slices(NHALF):
                                nc.tensor.matmul(
                                    ps[:, s:e], lhsT,
                                    ctx_sb[:, k, jh * NHALF + s : jh * NHALF + e],
                                    start=(k == 0), stop=(k == 1),
                                )
                        nc.vector.tensor_scalar_add(
                            out=k_sb[:, g, jh * NHALF : (jh + 1) * NHALF],
                            in0=ps, scalar1=b_sb["bk"][:, g : g + 1],
                        )

                # V transposed: [j_tile 128, 256] -> vT_sb bf16
                for jt in range(NJT):
                    ps = pp.tile([128, C], F32, tag="vp")
                    for k in range(2):
                        nc.tensor.matmul(
                            ps, ctx_sb[:, k, jt * 128 : (jt + 1) * 128],
                            w_sb["wvT"][:, k, :],
                            start=(k == 0), stop=(k == 1),
                        )
                    for h in range(NH):
                        nc.vector.tensor_copy(
                            out=vT_sb[:, jt, h * (HD + 1) : h * (HD + 1) + HD],
                            in_=ps[:, h * HD : (h + 1) * HD],
                        )

            # ---- attention (bf16 matmuls) ----
            with (
                tc.tile_pool(name="expp", bufs=20) as expp,
                tc.tile_pool(name="aonp", bufs=3) as aonp,
                tc.tile_pool(name="scp", bufs=2, space="PSUM") as scp,
                tc.tile_pool(name="avp", bufs=1, space="PSUM") as avp,
            ):
                for h in range(NH):
                    g, off = h // 2, (h % 2) * HD
                    kv_col = h * (HD + 1)
                    exp_tiles = []
                    for jt in range(NJT):
                        ps = scp.tile([128, NHALF], F32, tag="sc")
                        lhsT = k_sb[off : off + HD, g, jt * 128 : (jt + 1) * 128]
                        rhs = q_sb[off : off + HD, g, :]
                        for s, e in _mm_slices(NHALF):
                            nc.tensor.matmul(ps[:, s:e], lhsT, rhs[:, s:e],
                                             start=True, stop=True)
                        ex = expp.tile([128, NHALF], BF16, tag="exp")
                        nc.scalar.activation(out=ex, in_=ps, func=AF.Exp, scale=SCALE)
                        exp_tiles.append(ex)

                    for it in range(NHALF // 128):  # 9 query tiles of 128
                        av = avp.tile([128, HD + 1], F32, tag="av")
                        for jt in range(NJT):
                            nc.tensor.matmul(
                                av, exp_tiles[jt][:, it * 128 : (it + 1) * 128],
                                vT_sb[:, jt, kv_col : kv_col + HD + 1],
                                start=(jt == 0), stop=(jt == NJT - 1),
                            )
                        rden = small.tile([128, 1], F32, tag="rden")
                        nc.vector.reciprocal(out=rden, in_=av[:, HD : HD + 1])
                        aon = aonp.tile([128, HD], F32, tag="aon")
                        nc.vector.tensor_scalar_mul(
                            out=aon, in0=av[:, :HD], scalar1=rden
                        )
                        avt = avp.tile([HD, 128], F32, tag="avt")
                        nc.tensor.transpose(avt, aon, ident)
                        nc.vector.tensor_copy(
                            out=ao_sb[off : off + HD, g, it * 128 : (it + 1) * 128],
                            in_=avt,
                        )

            # ---- output projection + residual + GroupNorm ----
            with (
                tc.tile_pool(name="wop", bufs=2, space="PSUM") as wop,
                tc.tile_pool(name="gnp", bufs=1, space="PSUM") as gnp,
            ):
                for g in range(2):  # v bias (usually zero, kept for generality)
                    nc.vector.tensor_scalar_add(
                        out=ao_sb[:, g, :], in0=ao_sb[:, g, :],
                        scalar1=b_sb["bv"][:, g : g + 1],
                    )
                for g in range(2):
                    ps = wop.tile([128, NHALF], F32, tag="wo")
                    for k in range(2):
                        lhsT = w_sb["woT"][:, k, g * 128 : (g + 1) * 128]
                        for s, e in _mm_slices(NHALF):
                            nc.tensor.matmul(ps[:, s:e], lhsT, ao_sb[:, k, s:e],
                                             start=(k == 0), stop=(k == 1))
                    nc.vector.tensor_scalar_add(
                        out=y_sb[:, g, :], in0=ps, scalar1=b_sb["bo"][:, g : g + 1]
                    )
                    nc.vector.tensor_add(
                        out=y_sb[:, g, :], in0=y_sb[:, g, :], in1=xh_sb[:, g, :]
                    )
                    nc.vector.reduce_sum(
                        out=sums[:, g : g + 1], in_=y_sb[:, g, :],
                        axis=mybir.AxisListType.X,
                    )
                    nc.vector.tensor_mul(out=scr, in0=y_sb[:, g, :], in1=y_sb[:, g, :])
                    nc.vector.reduce_sum(
                        out=sumsq[:, g : g + 1], in_=scr, axis=mybir.AxisListType.X
                    )

                # pair AllReduce of per-channel (sum, sumsq)
                gn_in = dram.tile([C, 2], F32, tag="gnin", bufs=1)
                gn_out = dram.tile([C, 2], F32, tag="gnout", bufs=1)
                gn_in_r = gn_in.rearrange("(k p) s -> p k s", p=128)
                for k in range(2):
                    nc.sync.dma_start(out=gn_in_r[:, k, 0:1], in_=sums[:, k : k + 1])
                    nc.sync.dma_start(out=gn_in_r[:, k, 1:2], in_=sumsq[:, k : k + 1])
                nc.gpsimd.collective_compute(
                    "AllReduce", ALU.add,
                    replica_groups=[[0, 1], [2, 3], [4, 5], [6, 7]],
                    ins=[gn_in.opt()], outs=[gn_out.opt()],
                )
                gs_sb = small.tile([128, 2, 2], F32, tag="gs", bufs=1)
                nc.sync.dma_start(out=gs_sb, in_=gn_out.rearrange("(k p) s -> p k s", p=128))

                # group totals via 0/1 selection matmul: [16 local groups, (sum,sumsq)]
                gtot = small.tile([16, 2, 2], F32, tag="gtot", bufs=1)
                for k in range(2):
                    gp = gnp.tile([16, 2], F32, tag="gp")
                    nc.tensor.matmul(gp, gsel_sb, gs_sb[:, k, :],
                                     start=True, stop=True)
                    nc.vector.tensor_copy(out=gtot[:, k, :], in_=gp)
                mean_g = small.tile([16, 2], F32, tag="meang", bufs=1)
                var_g = small.tile([16, 2], F32, tag="varg", bufs=1)
                nc.scalar.mul(out=mean_g, in_=gtot[:, :, 0], mul=1.0 / GN_COUNT)
                nc.scalar.mul(out=var_g, in_=gtot[:, :, 1], mul=1.0 / GN_COUNT)
                m2 = small.tile([16, 2], F32, tag="m2", bufs=1)
                nc.vector.tensor_mul(out=m2, in0=mean_g, in1=mean_g)
                nc.vector.tensor_tensor(out=var_g, in0=var_g, in1=m2, op=ALU.subtract)
                nc.scalar.activation(out=var_g, in_=var_g, func=AF.Sqrt, bias=eps_sb)
                nc.vector.reciprocal(out=var_g, in_=var_g)  # rstd [16, 2]

                # broadcast group stats to channels: [128, 2] via gselT matmul
                mean_c = small.tile([128, 2], F32, tag="meanc", bufs=1)
                rstd_c = small.tile([128, 2], F32, tag="rstdc", bufs=1)
                for src, dst in ((mean_g, mean_c), (var_g, rstd_c)):
                    gp = gnp.tile([128, 2], F32, tag="gb")
                    nc.tensor.matmul(gp, gselT_sb, src, start=True, stop=True)
                    nc.vector.tensor_copy(out=dst, in_=gp)

                yr = yh_d.rearrange("(k p) i -> p k i", p=128)
                for g in range(2):
                    nc.vector.tensor_scalar(
                        out=y_sb[:, g, :], in0=y_sb[:, g, :],
                        scalar1=mean_c[:, g : g + 1], scalar2=rstd_c[:, g : g + 1],
                        op0=ALU.subtract, op1=ALU.mult,
                    )
                    nc.vector.tensor_scalar(
                        out=y_sb[:, g, :], in0=y_sb[:, g, :],
                        scalar1=b_sb["gamma"][:, g : g + 1],
                        scalar2=b_sb["beta"][:, g : g + 1],
                        op0=ALU.mult, op1=ALU.add,
                    )
                    for s, e in _mm_slices(NHALF):
                        nc.sync.dma_start(out=yr[:, g, s:e], in_=y_sb[:, g, s:e])

    _finalize(nc)
    return nc


def _get_nc():
    if "nc" not in _CACHE:
        _CACHE["nc"] = _build()
    return _CACHE["nc"]


def make_in_maps(x, context, Wq, bq, Wk, bk, Wv, bv, Wo, bo, gamma, beta):
    x = np.asarray(x, np.float32)
    context = np.asarray(context, np.float32)
    xr = np.ascontiguousarray(x.reshape(B, C, HW))
    cr = np.ascontiguousarray(context.reshape(B, C, HW))

    gsel = np.zeros((128, 16), np.float32)
    gsel[np.arange(128), np.arange(128) // GSIZE] = 1.0

    shared = {
        "wqT": np.ascontiguousarray(np.asarray(Wq, np.float32).T),
        "wkT": np.ascontiguousarray(np.asarray(Wk, np.float32).T),
        "wvT": np.ascontiguousarray(np.asarray(Wv, np.float32).T),
        "woT": np.ascontiguousarray(np.asarray(Wo, np.float32).T),
        "bq": np.asarray(bq, np.float32).reshape(C, 1),
        "bk": np.asarray(bk, np.float32).reshape(C, 1),
        "bv": np.asarray(bv, np.float32).reshape(C, 1),
        "bo": np.asarray(bo, np.float32).reshape(C, 1),
        "gamma": np.asarray(gamma, np.float32).reshape(C, 1),
        "beta": np.asarray(beta, np.float32).reshape(C, 1),
        "gsel": gsel,
        "gselT": np.ascontiguousarray(gsel.T),
    }
    in_maps = []
    for core in range(8):
        b, half = core // 2, core % 2
        m = dict(shared)
        m["xh"] = np.ascontiguousarray(xr[b, :, half * NHALF : (half + 1) * NHALF])
        m["xhr"] = m["xh"]
        m["ctx"] = cr[b]
        in_maps.append(m)
    return in_maps


def kernel(x, context, Wq, bq, Wk, bk, Wv, bv, Wo, bo, gamma, beta):
    in_maps = make_in_maps(
        x, context, Wq, bq, Wk, bk, Wv, bv, Wo, bo, gamma, beta
    )
    x = np.asarray(x, np.float32)

    nc = _get_nc()
    res = run_bass_kernel_spmd(nc, in_maps, core_ids=list(range(8)))

    out = np.empty((B, C, HW), np.float32)
    for core in range(8):
        b, half = core // 2, core % 2
        out[b, :, half * NHALF : (half + 1) * NHALF] = res.results[core]["yh"]
    return out.reshape(x.shape)
